# revision 40
# baseline (speedup 1.0000x reference)
"""3-layer GCN (CrystalGCN) on 8 TRN2 NeuronCores.

Strategy (graph/data parallel, nodes sharded by range):
  - 50000 nodes -> 6250/core (padded 6272 = 49 tiles of 128).
  - Edges (incl. self-loops) assigned to the core owning their dst.
  - Scatter-add via PE matmul: aggT[f, dst] += msgs[e, f].T @ S[e, dst],
    with the one-hot*dinv S tiles PRecomputed on host and streamed from
    DRAM over HWDGE (the on-chip DVE build was the v1 bottleneck).
  - gpsimd.dma_gather calls round-robin over 4 SWDGE queues so all four
    Q7 core pairs generate descriptors in parallel (v1 used one pair).
  - Layer 1: aggregate-first: z1 = (A@x) @ W1; h1 node-major -> AllGather.
  - Layer 2: keeps aggT f-major: z2T = W2.T @ agg2; h2T stays in SBUF,
    xw3 = (dinv*h2) @ W3 computed immediately (transform-first for L3),
    padded to 128 cols, AllGather (16x smaller than gathering h2).
  - Layer 3: gather xw3[src] (128-wide, 16 valid), scatter to agg3T
    [16, dst], PE-transpose, + b3, log_softmax.

Host preprocessing (numpy) builds index/metadata/S tensors; the device
kernel is static given the (fixed) edge distribution statistics.
"""
import numpy as np
import ml_dtypes

N = 50000
E = 800000
F_IN, F_HID, F_OUT = 128, 256, 10
F_OUT_P = 16
NCORES = 8
NSH = N // NCORES            # 6250
P = 128
NT = (NSH + P - 1) // P      # 49 node tiles per core
NSHP = NT * P                # 6272 padded shard rows
NROWS = NSHP * NCORES        # 50176 padded global rows
HI_OFF = 17408               # hi table = rows [17408, 50176), 32768 rows
LO_LIM = 32768
BATCH = 4                    # node tiles per gather pair

BF16 = ml_dtypes.bfloat16


def _wrap_idx16(vals):
    """dma_gather index layout: edge i -> [i%16, i//16], replicated to 8
    groups of 16 partitions (one copy per Q7 core)."""
    n = len(vals)
    assert n % 16 == 0
    blk = np.asarray(vals, dtype=np.int16).reshape(n // 16, 16).T
    return np.tile(blk, (8, 1))


def _preprocess(x, edge_index):
    """Build per-core gather/scatter metadata. Returns dict of host arrays."""
    x = np.asarray(x, dtype=np.float32)
    ei = np.asarray(edge_index, dtype=np.int64)
    src_all = np.concatenate([ei[0], np.arange(N, dtype=np.int64)])
    dst_all = np.concatenate([ei[1], np.arange(N, dtype=np.int64)])
    deg = np.bincount(dst_all, minlength=N).astype(np.float32)
    dinv = np.where(deg > 0, 1.0 / np.sqrt(deg), 0.0).astype(np.float32)

    # gather row numbering uses padded global rows
    gidx_all = (src_all // NSH) * NSHP + (src_all % NSH)

    # x gather source: dinv-prescaled, padded layout, bf16
    x_pad = np.zeros((NROWS, F_IN), dtype=BF16)
    xs = (x * dinv[:, None]).astype(BF16)
    for c in range(NCORES):
        x_pad[c * NSHP:c * NSHP + NSH] = xs[c * NSH:(c + 1) * NSH]

    # self-edges (added loops + natural src==dst) bypass the dma_gather
    # path: their contribution comes from a sequential own-rows load times
    # a diagonal S tile, so they cost no Q7 descriptor generation
    is_self = src_all == dst_all
    d_extra = dinv * np.bincount(dst_all[is_self], minlength=N)

    src_e = src_all[~is_self]
    dst_e = dst_all[~is_self]
    gidx_e = gidx_all[~is_self]

    core_of = dst_e // NSH
    tile_of = (dst_e % NSH) // P
    slot_of = (dst_e % NSH) % P

    # per (core, tile) edge lists sorted by gidx
    per = {}
    cnts = np.zeros((NCORES, NT), dtype=np.int64)
    lo_cap = np.zeros((NCORES, NT), dtype=np.int64)
    hi_cap = np.zeros((NCORES, NT), dtype=np.int64)
    order = np.lexsort((gidx_e, tile_of, core_of))
    g_sorted = gidx_e[order]
    slot_sorted = slot_of[order]
    dst_sorted = dst_e[order]
    key = core_of[order] * NT + tile_of[order]
    starts = np.searchsorted(key, np.arange(NCORES * NT))
    ends = np.searchsorted(key, np.arange(NCORES * NT), side="right")
    for c in range(NCORES):
        for t in range(NT):
            k = c * NT + t
            g = g_sorted[starts[k]:ends[k]]
            per[(c, t)] = (g, slot_sorted[starts[k]:ends[k]],
                           dst_sorted[starts[k]:ends[k]])
            cnts[c, t] = len(g)
            lo_cap[c, t] = np.searchsorted(g, LO_LIM)
            hi_cap[c, t] = len(g) - np.searchsorted(g, HI_OFF)

    tl_min = int(np.ceil((cnts - hi_cap).max() / P))
    tl_max = int(lo_cap.min() // P)
    assert tl_min <= tl_max, (tl_min, tl_max)
    # TL=8 makes each lo stream an exact multiple of the 1024-idx
    # dma_gather limit (8 tiles * 128)
    TL = int(np.clip(8, tl_min, tl_max))
    TH = int(np.ceil((cnts.max() - TL * P) / P))
    TNT = TL + TH

    TNT_S = TNT + 1          # edge tiles + trailing diagonal (self) tile
    cores = []
    for c in range(NCORES):
        s_host = np.zeros((P, NT * TNT_S * P), dtype=np.float32)
        lo_idx_parts = []
        hi_idx_parts = []
        for t in range(NT):
            g, sl, dd = per[(c, t)]
            nlo = TL * P
            glo, ghi = g[:nlo], g[nlo:]
            sllo, slhi = sl[:nlo], sl[nlo:]
            ddlo, ddhi = dd[:nlo], dd[nlo:]
            assert glo.max() < LO_LIM and (len(ghi) == 0 or ghi.min() >= HI_OFF)
            nhi = len(ghi)
            # hi dummies use idx 0 (valid row; S=0 nullifies them)
            hi_stream = np.zeros(TH * P, dtype=np.int64)
            hi_stream[:nhi] = ghi - HI_OFF
            lo_idx_parts.append(_wrap_idx16(glo))
            hi_idx_parts.append(_wrap_idx16(hi_stream))
            mlo = np.arange(nlo)
            s_host[mlo % P, (t * TNT_S + mlo // P) * P + sllo] = dinv[ddlo]
            mhi = np.arange(nhi)
            s_host[mhi % P, (t * TNT_S + TL + mhi // P) * P + slhi] = dinv[ddhi]
            # diagonal self tile
            base = c * NSH + t * P
            ns = min(P, NSH - t * P)
            s = np.arange(ns)
            s_host[s, (t * TNT_S + TNT) * P + s] = d_extra[base:base + ns]
        idx_lo = np.concatenate(lo_idx_parts, axis=1)
        idx_hi = np.concatenate(hi_idx_parts, axis=1)

        dinv_node = np.zeros((P, NT), dtype=np.float32)
        loc = np.arange(NSH)
        dinv_node[loc % P, loc // P] = dinv[c * NSH:(c + 1) * NSH]

        cores.append({"idx_lo": idx_lo, "idx_hi": idx_hi,
                      "s_dram": s_host.astype(BF16),
                      "dinv_node": dinv_node,
                      "x_own": x_pad[c * NSHP:(c + 1) * NSHP]})

    return {
        "x_pad": x_pad, "cores": cores, "TL": TL, "TH": TH, "TNT": TNT,
    }


def _build_program(TL, TH, TNT):
    import concourse.bass as bass
    from concourse import bacc
    import concourse.mybir as mybir
    from concourse.tile import TileContext

    dt = mybir.dt
    Alu = mybir.AluOpType
    Act = mybir.ActivationFunctionType
    TNT_S = TNT + 1
    TE = NT * TNT_S

    nc = bacc.Bacc(num_devices=NCORES, num_swdge_queues=4)
    x_pad = nc.dram_tensor("x_pad", [NROWS, F_IN], dt.bfloat16, kind="ExternalInput")
    x_own = nc.dram_tensor("x_own", [NSHP, F_IN], dt.bfloat16, kind="ExternalInput")
    idx_lo = nc.dram_tensor("idx_lo", [P, NT * TL * 8], dt.int16, kind="ExternalInput")
    idx_hi = nc.dram_tensor("idx_hi", [P, NT * TH * 8], dt.int16, kind="ExternalInput")
    s_dram = nc.dram_tensor("s_dram", [P, TE * P], dt.bfloat16, kind="ExternalInput")
    meta = nc.dram_tensor("meta", [P, NT], dt.float32, kind="ExternalInput")
    # wts: W1 [0:256] | W2c fi0fo0,fi0fo1,fi1fo0,fi1fo1 [256:768]
    #      | W3c fi0,fi1 [768:800] | identity [800:928]
    wts = nc.dram_tensor("wts", [P, 928], dt.bfloat16, kind="ExternalInput")
    # bias: b1 row-replicated [0:256] | b2T per-partition [256:258]
    #       | b3 row-replicated padded [258:274] | identity f32 [274:402]
    bias = nc.dram_tensor("bias", [P, 402], dt.float32, kind="ExternalInput")
    out_d = nc.dram_tensor("out", [NSHP, F_OUT_P], dt.float32, kind="ExternalOutput")

    qctr = [0]

    def next_q():
        q = qctr[0] % 4
        qctr[0] += 1
        return q

    with TileContext(nc) as tc:
        with tc.tile_pool(name="const", bufs=1) as cpool, \
             tc.tile_pool(name="msgs", bufs=2) as mpool, \
             tc.tile_pool(name="stiles", bufs=3) as spool, \
             tc.tile_pool(name="work", bufs=3) as wpool, \
             tc.tile_pool(name="big", bufs=1) as bigpool, \
             tc.tile_pool(name="ps", bufs=1, space="PSUM") as pspool, \
             tc.tile_pool(name="dram", bufs=1, space="DRAM") as dpool:

            idxlo_sb = cpool.tile([P, NT * TL * 8], dt.int16)
            nc.sync.dma_start(out=idxlo_sb[:], in_=idx_lo[:])
            idxhi_sb = cpool.tile([P, NT * TH * 8], dt.int16)
            nc.sync.dma_start(out=idxhi_sb[:], in_=idx_hi[:])
            meta_sb = cpool.tile([P, NT], dt.float32)
            nc.sync.dma_start(out=meta_sb[:], in_=meta[:])
            wts_sb = cpool.tile([P, 928], dt.bfloat16)
            nc.sync.dma_start(out=wts_sb[:], in_=wts[:])
            bias_sb = cpool.tile([P, 402], dt.float32)
            nc.sync.dma_start(out=bias_sb[:], in_=bias[:])

            h1_shard = dpool.tile([NSHP, F_HID], dt.bfloat16)
            xw3_shard = dpool.tile([NSHP, P], dt.bfloat16)
            h1_full = dpool.tile([NROWS, F_HID], dt.bfloat16, addr_space="Shared")
            xw3_full = dpool.tile([NROWS, P], dt.bfloat16, addr_space="Shared")

            w1_ap = wts_sb[:, 0:256]
            w2c = {(0, 0): wts_sb[:, 256:384], (0, 1): wts_sb[:, 384:512],
                   (1, 0): wts_sb[:, 512:640], (1, 1): wts_sb[:, 640:768]}
            w3c = {0: wts_sb[:, 768:784], 1: wts_sb[:, 784:800]}
            b1_ap = bias_sb[:, 0:256]
            b2T_ap = bias_sb[:, 256:258]
            b3_ap = bias_sb[:, 258:274]
            ident_ap = bias_sb[:, 274:402]

            # persistent h2T storage: [P, NT, 2, P] bf16
            h2T = bigpool.tile([P, NT, 2, P], dt.bfloat16)

            MBUFS = 6

            def gathers(l, nt, gsrc, F_in):
                sfx = "A" if F_in == P else "B"
                msl = mpool.tile([P, TL, F_in], dt.bfloat16,
                                 tag=f"msl{sfx}", bufs=MBUFS,
                                 name=f"msl_{l}_{nt}")
                msh = mpool.tile([P, TH, F_in], dt.bfloat16,
                                 tag=f"msh{sfx}", bufs=MBUFS,
                                 name=f"msh_{l}_{nt}")
                if nt < MBUFS and l in (1, 2):
                    # first use of this slot: zero so -1-trimmed tail slots
                    # hold finite values (S=0 nullifies them in the matmul)
                    nc.vector.memset(msh[:], 0.0)
                nc.gpsimd.dma_gather(
                    out_ap=msl[:],
                    in_ap=gsrc[0:LO_LIM, :],
                    idxs_ap=idxlo_sb[:, nt * TL * 8:(nt + 1) * TL * 8],
                    num_idxs=TL * P, num_idxs_reg=TL * P,
                    elem_size=F_in, queue_num=next_q())
                for off in range(0, TH, 8):
                    ct = min(8, TH - off)
                    nc.gpsimd.dma_gather(
                        out_ap=msh[:, off:off + ct, :],
                        in_ap=gsrc[HI_OFF:HI_OFF + LO_LIM, :],
                        idxs_ap=idxhi_sb[:, nt * TH * 8 + off * 8:
                                         nt * TH * 8 + (off + ct) * 8],
                        num_idxs=ct * P, num_idxs_reg=ct * P,
                        elem_size=F_in, queue_num=next_q())
                return msl, msh

            def load_s(l, nt):
                s_t = spool.tile([P, TNT_S * P], dt.bfloat16, tag="s_t",
                                 bufs=4, name=f"s_{l}_{nt}")
                nc.sync.dma_start(out=s_t[:],
                                  in_=s_dram[:, nt * TNT_S * P:
                                             (nt + 1) * TNT_S * P])
                return s_t

            def load_own(l, nt, src_d, F_in):
                sfx = "A" if F_in == P else "B"
                own = wpool.tile([P, F_in], dt.bfloat16, tag=f"own{sfx}",
                                 bufs=3, name=f"own_{l}_{nt}")
                nc.sync.dma_start(out=own[:],
                                  in_=src_d[nt * P:(nt + 1) * P, :])
                return own

            def scatter(l, nt, s_t, msl, msh, own, nch, mcols):
                """aggT[f, dst] accumulation over TNT message tiles plus the
                diagonal self tile. nch feature chunks of width mcols."""
                aggps = [pspool.tile([P, P], dt.float32, space="PSUM",
                                     tag=f"agg{fc}", bufs=2,
                                     name=f"agg_{l}_{nt}_{fc}")
                         for fc in range(nch)]
                for j in range(TNT_S):
                    if j < TL:
                        m_ap = msl[:, j, :]
                    elif j < TNT:
                        m_ap = msh[:, j - TL, :]
                    else:
                        m_ap = own[:]
                    for fc in range(nch):
                        nc.tensor.matmul(
                            aggps[fc][0:mcols, :] if mcols < P else aggps[fc][:],
                            lhsT=m_ap[:, fc * mcols:(fc + 1) * mcols],
                            rhs=s_t[:, j * P:(j + 1) * P],
                            start=(j == 0), stop=(j == TNT_S - 1))
                return aggps

            # ---------------- Layer 1 ----------------
            for nt in range(NT):
                msl, msh = gathers(1, nt, x_pad, F_IN)
                s_t = load_s(1, nt)
                own = load_own(1, nt, x_own, F_IN)
                aggps = scatter(1, nt, s_t, msl, msh, own, 1, P)
                aggsb = wpool.tile([P, P], dt.bfloat16, tag="aggsb0",
                                   bufs=3, name=f"aggsb_1_{nt}")
                nc.vector.tensor_copy(out=aggsb[:], in_=aggps[0][:])
                zps = pspool.tile([P, 256], dt.float32, space="PSUM",
                                  tag="z", bufs=2, name=f"z_1_{nt}")
                nc.tensor.matmul(zps[:], lhsT=aggsb[:], rhs=w1_ap,
                                 start=True, stop=True)
                tmp = wpool.tile([P, 256], dt.float32, tag="tmp",
                                 bufs=3, name=f"tmp_1_{nt}")
                nc.vector.tensor_tensor(out=tmp[:], in0=zps[:],
                                        in1=b1_ap, op=Alu.add)
                h1sb = wpool.tile([P, 256], dt.bfloat16, tag="h1sb",
                                  bufs=3, name=f"h1sb_{nt}")
                nc.scalar.activation(
                    out=h1sb[:], in_=tmp[:], func=Act.Relu,
                    scale=meta_sb[:, nt:nt + 1])
                nc.sync.dma_start(out=h1_shard[nt * P:(nt + 1) * P, :],
                                  in_=h1sb[:])

            nc.gpsimd.collective_compute(
                "AllGather", mybir.AluOpType.bypass,
                replica_groups=[list(range(NCORES))],
                ins=[h1_shard[:].opt()], outs=[h1_full[:].opt()])

            # ---------------- Layer 2 (+ xw3 for transform-first L3) -------
            for nt in range(NT):
                msl, msh = gathers(2, nt, h1_full, F_HID)
                s_t = load_s(2, nt)
                own = load_own(2, nt, h1_shard, F_HID)
                aggps = scatter(2, nt, s_t, msl, msh, own, 2, P)
                aggsb = []
                for fc in range(2):
                    a = wpool.tile([P, P], dt.bfloat16, tag=f"aggsb{fc}",
                                   bufs=3, name=f"aggsb_2_{nt}_{fc}")
                    nc.vector.tensor_copy(out=a[:], in_=aggps[fc][:])
                    aggsb.append(a)
                for fo in range(2):
                    z2t = pspool.tile([P, P], dt.float32, space="PSUM",
                                      tag="z", bufs=2, name=f"z2t_{nt}_{fo}")
                    for fi in range(2):
                        nc.tensor.matmul(z2t[:], lhsT=w2c[(fi, fo)],
                                         rhs=aggsb[fi][:],
                                         start=(fi == 0), stop=(fi == 1))
                    nc.scalar.activation(
                        out=h2T[:, nt, fo, :], in_=z2t[:], func=Act.Relu,
                        bias=b2T_ap[:, fo:fo + 1])
                xw3ps = pspool.tile([P, F_OUT_P], dt.float32, space="PSUM",
                                    tag="xw3", bufs=2, name=f"xw3_{nt}")
                for fi in range(2):
                    nc.tensor.matmul(xw3ps[:], lhsT=h2T[:, nt, fi, :],
                                     rhs=w3c[fi],
                                     start=(fi == 0), stop=(fi == 1))
                xw3sb = wpool.tile([P, P], dt.bfloat16, tag="xw3sb",
                                   bufs=3, name=f"xw3sb_{nt}")
                nc.vector.memset(xw3sb[:, F_OUT_P:P], 0.0)
                nc.scalar.activation(
                    out=xw3sb[:, 0:F_OUT_P], in_=xw3ps[:], func=Act.Copy,
                    scale=meta_sb[:, nt:nt + 1])
                nc.sync.dma_start(out=xw3_shard[nt * P:(nt + 1) * P, :],
                                  in_=xw3sb[:])

            nc.gpsimd.collective_compute(
                "AllGather", mybir.AluOpType.bypass,
                replica_groups=[list(range(NCORES))],
                ins=[xw3_shard[:].opt()], outs=[xw3_full[:].opt()])

            # ---------------- Layer 3 ----------------
            for nt in range(NT):
                msl, msh = gathers(3, nt, xw3_full, P)
                s_t = load_s(3, nt)
                own = load_own(3, nt, xw3_shard, P)
                aggps = scatter(3, nt, s_t, msl, msh, own, 1, F_OUT_P)
                a3sb = wpool.tile([P, P], dt.float32, tag="a3sb",
                                  bufs=2, name=f"a3sb_{nt}")
                nc.scalar.copy(out=a3sb[0:F_OUT_P, :],
                               in_=aggps[0][0:F_OUT_P, :])
                trp = pspool.tile([P, F_OUT_P], dt.float32, space="PSUM",
                                  tag="xw3", bufs=2, name=f"trp_{nt}")
                nc.tensor.transpose(trp[:], in_=a3sb[0:F_OUT_P, :],
                                    identity=ident_ap[0:F_OUT_P, 0:F_OUT_P])
                z3 = wpool.tile([P, F_OUT_P], dt.float32, tag="tmp",
                                bufs=3, name=f"z3_{nt}")
                nc.vector.tensor_tensor(out=z3[:], in0=trp[:],
                                        in1=b3_ap, op=Alu.add)
                mx = wpool.tile([P, 1], dt.float32, tag="mx",
                                bufs=3, name=f"mx_{nt}")
                nc.vector.tensor_reduce(
                    out=mx[:], in_=z3[:, 0:F_OUT],
                    axis=mybir.AxisListType.X, op=Alu.max, negate=True)
                ex = wpool.tile([P, F_OUT], dt.float32, tag="ex",
                                bufs=3, name=f"ex_{nt}")
                nc.scalar.activation(out=ex[:], in_=z3[:, 0:F_OUT],
                                     func=Act.Exp, bias=mx[:])
                sm = wpool.tile([P, 1], dt.float32, tag="sm",
                                bufs=3, name=f"sm_{nt}")
                nc.vector.tensor_reduce(
                    out=sm[:], in_=ex[:],
                    axis=mybir.AxisListType.X, op=Alu.add)
                ls = wpool.tile([P, 1], dt.float32, tag="ls",
                                bufs=3, name=f"ls_{nt}")
                nc.scalar.activation(out=ls[:], in_=sm[:], func=Act.Ln)
                # c = mx - ls  (mx already holds -max)
                cb = wpool.tile([P, 1], dt.float32, tag="nls",
                                bufs=3, name=f"cb_{nt}")
                nc.scalar.activation(out=cb[:], in_=ls[:], func=Act.Identity,
                                     scale=-1.0, bias=mx[:])
                ob = wpool.tile([P, F_OUT_P], dt.float32, tag="ob",
                                bufs=3, name=f"ob_{nt}")
                nc.vector.memset(ob[:, F_OUT:F_OUT_P], 0.0)
                nc.scalar.activation(out=ob[:, 0:F_OUT], in_=z3[:, 0:F_OUT],
                                     func=Act.Identity, bias=cb[:])
                nc.sync.dma_start(out=out_d[nt * P:(nt + 1) * P, :],
                                  in_=ob[:])

    nc.finalize()
    return nc


_CACHE = {}


def kernel(x, edge_index, W1, b1, W2, b2, W3, b3):
    from concourse.bass_utils import run_bass_kernel_spmd

    prep = _preprocess(x, edge_index)
    TL, TH, TNT = prep["TL"], prep["TH"], prep["TNT"]

    key = (TL, TH)
    if key not in _CACHE:
        _CACHE[key] = _build_program(TL, TH, TNT)
    nc = _CACHE[key]

    W1 = np.asarray(W1, np.float32)
    W2 = np.asarray(W2, np.float32)
    W3 = np.asarray(W3, np.float32)
    wts = np.zeros((P, 928), dtype=BF16)
    wts[:, 0:256] = W1.astype(BF16)
    for fi in range(2):
        for fo in range(2):
            wts[:, 256 + (fi * 2 + fo) * 128:256 + (fi * 2 + fo + 1) * 128] = \
                W2[fi * 128:(fi + 1) * 128, fo * 128:(fo + 1) * 128].astype(BF16)
    wts[:, 768:778] = W3[0:128].astype(BF16)
    wts[:, 784:794] = W3[128:256].astype(BF16)
    bias = np.zeros((P, 402), dtype=np.float32)
    bias[:, 0:256] = np.asarray(b1, np.float32)[None, :]
    b2 = np.asarray(b2, np.float32)
    bias[:, 256] = b2[0:128]
    bias[:, 257] = b2[128:256]
    bias[:, 258:268] = np.asarray(b3, np.float32)[None, :]
    bias[:, 274:402] = np.eye(P, dtype=np.float32)

    in_maps = []
    for c in range(NCORES):
        m = dict(prep["cores"][c])
        m["x_pad"] = prep["x_pad"]
        m["meta"] = m.pop("dinv_node")
        m["wts"] = wts
        m["bias"] = bias
        in_maps.append(m)

    res = run_bass_kernel_spmd(nc, in_maps, core_ids=list(range(NCORES)))
    global LAST_EXEC_NS, LAST_TRACE
    LAST_EXEC_NS = getattr(res, "exec_time_ns", None)
    iat = getattr(res, "instructions_and_trace", None)
    LAST_TRACE = iat[1] if iat else None
    out = np.zeros((N, F_OUT), dtype=np.float32)
    for c in range(NCORES):
        out[c * NSH:(c + 1) * NSH] = res.results[c]["out"][:NSH, :F_OUT]
    return out


LAST_EXEC_NS = None
LAST_TRACE = None
